# revision 1
# baseline (speedup 1.0000x reference)
"""Trainium2 Bass kernel for GQA attention with RoPE (nn_Attention).

Reference (B=2, TQ=TKV=2048, D=2048, HQ=16, HKV=4, HD=128):
    q = Xq @ Wq; k = Xkv @ Wk; v = Xkv @ Wv
    q, k = rope(q, q_pos), rope(k, kv_pos)
    out = (causal_softmax(q k^T / sqrt(HD)) v) @ Wo   (kv head h//4 serves q head h)

Sharding: 8 cores = 2 batches x 4 query shards. Each core owns 8 interleaved
64-row query chunks (chunk i of core j is 4i + (j if i even else 3-j), which
balances the causal work exactly) and all 16 heads for those rows, so the
output projection needs no inter-core reduction. K/V projections are sharded
over the sequence (512 rows per core) and exchanged with AllGathers within
each batch group of 4 cores.

Device layout: scores are computed transposed (S^T[kv, q]) so attention*V
needs no transposes. The SPMD NEFF is identical on all cores, so the causal
block schedule is the conservative core-independent one: kv block g (128
rows) runs against query columns [64*(g//2) : 512]; only the first 64-col
sub-block's validity differs per core and is handled by a multiplicative
0/1 mask shipped as data. Softmax denominators accumulate in PSUM via
select-column ones matmuls; normalization is folded into the PSUM->SBUF
copy of the context.
"""
import numpy as np
import ml_dtypes

B = 2
T = 2048
D = 2048
HQ = 16
HKV = 4
HD = 128
HALF = HD // 2
N_CORES = 8
QROWS = 512
KVSH = 512
SCALE = 1.0 / float(np.sqrt(HD))
MAX_TIMESCALE = 10000.0

# 8 chunks of 64 query rows per core; chunk i lives in [4i, 4i+3]
CHUNKS = [[4 * i + (j if i % 2 == 0 else 3 - j) for i in range(8)]
          for j in range(4)]

_CACHE = {}


def _build():
    import concourse.mybir as mybir
    import concourse.tile as tile
    from concourse import bacc

    bf = mybir.dt.bfloat16
    f32 = mybir.dt.float32
    f32r = mybir.dt.float32r

    nc = bacc.Bacc("TRN2", target_bir_lowering=False, debug=False,
                   num_devices=N_CORES)

    # activations/weights arrive pre-arranged so SBUF loads are contiguous:
    # [128 partition, 16 k-tiles * cols]
    xqT = nc.dram_tensor("xqT", [128, 16 * QROWS], bf, kind="ExternalInput").ap()
    xkvT = nc.dram_tensor("xkvT", [128, 16 * KVSH], bf, kind="ExternalInput").ap()
    wq = nc.dram_tensor("wq", [128, HQ * 16 * HD], bf, kind="ExternalInput").ap()
    wk = nc.dram_tensor("wk", [128, 16 * HKV * HD], bf, kind="ExternalInput").ap()
    wv = nc.dram_tensor("wv", [128, 16 * HKV * HD], bf, kind="ExternalInput").ap()
    wo = nc.dram_tensor("wo", [HQ * HD, D], bf, kind="ExternalInput").ap()
    cosq = nc.dram_tensor("cosq", [HD, QROWS], bf, kind="ExternalInput").ap()
    sinq = nc.dram_tensor("sinq", [HD, QROWS], bf, kind="ExternalInput").ap()
    coskv = nc.dram_tensor("coskv", [HD, KVSH], bf, kind="ExternalInput").ap()
    sinkv = nc.dram_tensor("sinkv", [HD, KVSH], bf, kind="ExternalInput").ap()
    dmask = nc.dram_tensor("dmask", [16, 128, 256], bf, kind="ExternalInput").ap()
    selbc = nc.dram_tensor("selbc", [4, 4 * HD], mybir.dt.float32, kind="ExternalInput").ap()
    out = nc.dram_tensor("out", [QROWS, D], f32, kind="ExternalOutput").ap()

    Exp = mybir.ActivationFunctionType.Exp

    with tile.TileContext(nc) as tc:
        with tc.tile_pool(name="dram", bufs=1, space="DRAM") as dram, \
             tc.tile_pool(name="persist", bufs=1) as persist:

            # ---------------- persistent SBUF tiles ----------------
            qt_sb = [persist.tile([HD, QROWS], bf, name=f"qt{h}") for h in range(HQ)]
            kt_sb = [persist.tile([HD, T], bf, name=f"ktg{h}") for h in range(HKV)]
            v_sb = [persist.tile([128, 16 * HD], bf, name=f"vg{h}") for h in range(HKV)]
            ctxn_sb = [persist.tile([HD, QROWS], bf, name=f"ctxn{h}") for h in range(HQ)]
            mask_sb = persist.tile([128, 16 * 256], bf, name="mask_sb")
            cq = persist.tile([HD, QROWS], bf, name="cq")
            sq = persist.tile([HD, QROWS], bf, name="sq")
            ckv = persist.tile([HD, KVSH], bf, name="ckv")
            skv = persist.tile([HD, KVSH], bf, name="skv")
            # final-sums lhsT: sel4r[qh] = [128, 4] f32r, only column qh ones
            sel4_f = [persist.tile([128, 4], f32, name=f"sel4f_{q}") for q in range(4)]
            sel4r = [persist.tile([128, 4], f32r, name=f"sel4r_{q}") for q in range(4)]
            # bcast lhsT: sel128r[qh] = [4, 128] f32r with only row qh ones
            sel128_f = persist.tile([4, 4 * HD], f32, name="sel128_f")
            sel128r_all = persist.tile([4, 4 * HD], f32r, name="sel128r_all")
            sel128r = [sel128r_all[:, q * HD:(q + 1) * HD] for q in range(4)]

            nc.sync.dma_start(cq[:], cosq)
            nc.sync.dma_start(sq[:], sinq)
            nc.sync.dma_start(ckv[:], coskv)
            nc.sync.dma_start(skv[:], sinkv)
            nc.sync.dma_start(mask_sb.rearrange("p (g c) -> p g c", g=16),
                              dmask.rearrange("g p c -> p g c"))
            nc.sync.dma_start(sel128_f[:], selbc)
            nc.vector.tensor_copy(sel128r_all[:], sel128_f[:])
            for q in range(4):
                nc.vector.memset(sel4_f[q][:], 0.0)
                nc.vector.memset(sel4_f[q][:, q:q + 1], 1.0)
                nc.vector.tensor_copy(sel4r[q][:], sel4_f[q][:])

            # K bounce [4h][4b][128p][128c]; V bounce [4b][128p][512c]
            bk_in = dram.tile([262144], bf, name="bk_in")
            bk_out = dram.tile([4 * 262144], bf, name="bk_out")
            bv_in = dram.tile([262144], bf, name="bv_in")
            bv_out = dram.tile([4 * 262144], bf, name="bv_out")

            # ---------------- phase 1: projections ----------------
            with tc.tile_pool(name="acts", bufs=1) as acts, \
                 tc.tile_pool(name="wpool", bufs=4) as wpool, \
                 tc.tile_pool(name="rope_tmp", bufs=4) as rtmp, \
                 tc.tile_pool(name="proj_ps", bufs=3, space="PSUM") as proj_ps, \
                 tc.tile_pool(name="raw", bufs=3) as raw:

                xkv_sb = acts.tile([128, 16 * KVSH], bf, name="xkv_sb")
                nc.sync.dma_start(xkv_sb[:], xkvT)
                wk_sb = acts.tile([128, 16 * HKV * HD], bf, name="wk_sb")
                nc.sync.dma_start(wk_sb[:], wk)
                wv_sb = acts.tile([128, 16 * HKV * HD], bf, name="wv_sb")
                nc.sync.dma_start(wv_sb[:], wv)
                xq_sb = acts.tile([128, 16 * QROWS], bf, name="xq_sb")
                nc.sync.dma_start(xq_sb[:], xqT)
                # K^T shard per kv head: [hd, 512 kv] -> rope -> bounce
                for h in range(HKV):
                    ps = proj_ps.tile([HD, KVSH], f32, tag="proj", name=f"kps{h}")
                    for kt in range(16):
                        nc.tensor.matmul(
                            ps[:],
                            wk_sb[:, kt * 512 + h * HD:kt * 512 + (h + 1) * HD],
                            xkv_sb[:, kt * KVSH:(kt + 1) * KVSH],
                            start=(kt == 0), stop=(kt == 15))
                    kraw = raw.tile([HD, KVSH], bf, tag="raw", name=f"kraw{h}")
                    nc.scalar.copy(kraw[:], ps[:])
                    ktr = raw.tile([HD, KVSH], bf, tag="roped", name=f"ktr{h}")
                    t1 = rtmp.tile([HALF, KVSH], bf, tag="t1", name=f"kt1_{h}")
                    t2 = rtmp.tile([HALF, KVSH], bf, tag="t2", name=f"kt2_{h}")
                    nc.vector.tensor_mul(t1[:], kraw[0:HALF, :], ckv[0:HALF, :])
                    nc.vector.tensor_mul(t2[:], kraw[HALF:HD, :], skv[HALF:HD, :])
                    nc.vector.tensor_sub(ktr[0:HALF, :], t1[:], t2[:])
                    t3 = rtmp.tile([HALF, KVSH], bf, tag="t1", name=f"kt3_{h}")
                    t4 = rtmp.tile([HALF, KVSH], bf, tag="t2", name=f"kt4_{h}")
                    nc.vector.tensor_mul(t3[:], kraw[0:HALF, :], skv[0:HALF, :])
                    nc.vector.tensor_mul(t4[:], kraw[HALF:HD, :], ckv[HALF:HD, :])
                    nc.vector.tensor_add(ktr[HALF:HD, :], t3[:], t4[:])
                    nc.sync.dma_start(
                        bk_in[h * 65536:(h + 1) * 65536].rearrange(
                            "(b p c) -> p b c", b=4, p=HD),
                        ktr.rearrange("p (b c) -> p b c", b=4))

                nc.gpsimd.collective_compute(
                    "AllGather", mybir.AluOpType.bypass,
                    replica_groups=[[0, 1, 2, 3], [4, 5, 6, 7]],
                    ins=[bk_in.opt()], outs=[bk_out.opt()])

                # V shard per kv block: [128 kv, 4h*128] -> bounce
                for b in range(4):
                    ps = proj_ps.tile([128, HKV * HD], f32, tag="proj", name=f"vps{b}")
                    for kt in range(16):
                        nc.tensor.matmul(
                            ps[:],
                            xkv_sb[:, kt * KVSH + b * 128:kt * KVSH + (b + 1) * 128],
                            wv_sb[:, kt * 512:(kt + 1) * 512],
                            start=(kt == 0), stop=(kt == 15))
                    vsh = raw.tile([128, HKV * HD], bf, tag="raw", name=f"vsh{b}")
                    nc.scalar.copy(vsh[:], ps[:])
                    nc.sync.dma_start(
                        bv_in[b * 65536:(b + 1) * 65536]
                        .rearrange("(p c) -> p c", p=128),
                        vsh[:])

                nc.gpsimd.collective_compute(
                    "AllGather", mybir.AluOpType.bypass,
                    replica_groups=[[0, 1, 2, 3], [4, 5, 6, 7]],
                    ins=[bv_in.opt()], outs=[bv_out.opt()])

                # Q projection per head: Q^T [hd, 512] -> rope
                for h in range(HQ):
                    wq_sb = wpool.tile([128, 16 * HD], bf, tag="wq", name=f"wqsb{h}")
                    nc.sync.dma_start(wq_sb[:], wq[:, h * 2048:(h + 1) * 2048])
                    ps = proj_ps.tile([HD, QROWS], f32, tag="proj", name=f"qps{h}")
                    for kt in range(16):
                        nc.tensor.matmul(
                            ps[:],
                            wq_sb[:, kt * HD:(kt + 1) * HD],
                            xq_sb[:, kt * QROWS:(kt + 1) * QROWS],
                            start=(kt == 0), stop=(kt == 15))
                    qraw = raw.tile([HD, QROWS], bf, tag="raw", name=f"qraw{h}")
                    nc.scalar.copy(qraw[:], ps[:])
                    t1 = rtmp.tile([HALF, QROWS], bf, tag="t1", name=f"qt1_{h}")
                    t2 = rtmp.tile([HALF, QROWS], bf, tag="t2", name=f"qt2_{h}")
                    nc.vector.tensor_mul(t1[:], qraw[0:HALF, :], cq[0:HALF, :])
                    nc.vector.tensor_mul(t2[:], qraw[HALF:HD, :], sq[HALF:HD, :])
                    nc.vector.tensor_sub(qt_sb[h][0:HALF, :], t1[:], t2[:])
                    t3 = rtmp.tile([HALF, QROWS], bf, tag="t1", name=f"qt3_{h}")
                    t4 = rtmp.tile([HALF, QROWS], bf, tag="t2", name=f"qt4_{h}")
                    nc.vector.tensor_mul(t3[:], qraw[0:HALF, :], sq[0:HALF, :])
                    nc.vector.tensor_mul(t4[:], qraw[HALF:HD, :], cq[HALF:HD, :])
                    nc.vector.tensor_add(qt_sb[h][HALF:HD, :], t3[:], t4[:])

            # gathered K^T and V -> SBUF
            for h in range(HKV):
                for r in range(4):
                    nc.sync.dma_start(
                        kt_sb[h][:, r * 512:(r + 1) * 512].rearrange(
                            "p (b c) -> p b c", b=4),
                        bk_out[r * 262144 + h * 65536:
                               r * 262144 + (h + 1) * 65536].rearrange(
                            "(b p c) -> p b c", b=4, p=HD))
                    nc.sync.dma_start(
                        v_sb[h][:, r * 512:(r + 1) * 512].rearrange(
                            "p (b c) -> p b c", b=4),
                        bv_out[r * 262144:(r + 1) * 262144].rearrange(
                            "(b p cc) -> p b cc", b=4, p=128)
                        [:, :, h * HD:(h + 1) * HD])

            # output-projection weights prefetch (overlaps attention)
            wop_cm = tc.tile_pool(name="wo_pool", bufs=16)
            wop = wop_cm.__enter__()
            wo_sb = []
            for h in range(HQ):
                t = wop.tile([HD, D], bf, tag="wo", name=f"wosb{h}")
                nc.sync.dma_start(t[:], wo[h * HD:(h + 1) * HD, :])
                wo_sb.append(t)

            # ---------------- phase 2: attention ----------------
            with tc.tile_pool(name="score_ps", bufs=2, space="PSUM") as score_ps, \
                 tc.tile_pool(name="ctx_ps", bufs=4, space="PSUM") as ctx_ps, \
                 tc.tile_pool(name="sum_ps", bufs=1, space="PSUM") as sum_ps, \
                 tc.tile_pool(name="bc_ps", bufs=1, space="PSUM") as bc_ps, \
                 tc.tile_pool(name="exp_pool", bufs=10) as epool, \
                 tc.tile_pool(name="acc_pool", bufs=8) as apool, \
                 tc.tile_pool(name="norm_pool", bufs=2) as npool:

                for kvh in range(HKV):
                    ctx = [ctx_ps.tile([HD, QROWS], f32, tag="ctx", name=f"ctx{kvh}_{i}") for i in range(4)]
                    acc = [apool.tile([HD, QROWS], mybir.dt.float32r, tag="acc",
                                      name=f"acc{kvh}_{i}") for i in range(4)]
                    sums = sum_ps.tile([4, QROWS], f32, tag="sums", name=f"sums{kvh}")
                    for g in range(16):
                        off = 64 * (g // 2)
                        ng = QROWS - off
                        exps = {}
                        for qh in range(4):
                            h = kvh * 4 + qh
                            sc = score_ps.tile([HD, 512], f32, tag="sc", name=f"sc{kvh}_{g}_{qh}")
                            nc.tensor.matmul(
                                sc[:, 0:ng],
                                kt_sb[kvh][:, g * 128:(g + 1) * 128],
                                qt_sb[h][:, off:QROWS],
                                start=True, stop=True)
                            et = epool.tile([128, 512], bf, tag="exp",
                                            name=f"et{kvh}_{g}_{qh}")
                            nc.scalar.activation(et[:, 0:ng], sc[:, 0:ng], Exp,
                                                 scale=SCALE)
                            nc.vector.tensor_mul(
                                et[:, 0:64], et[:, 0:64],
                                mask_sb[:, g * 256:g * 256 + 64])
                            exps[qh] = et
                        for qh in range(4):
                            with nc.allow_low_precision(reason="f32r softmax sums"):
                                if g == 0:
                                    nc.vector.tensor_copy(acc[qh][:], exps[qh][:])
                                else:
                                    nc.vector.tensor_add(
                                        acc[qh][:, off:QROWS],
                                        acc[qh][:, off:QROWS],
                                        exps[qh][:, 0:ng])
                        for qh in range(4):
                            nc.tensor.matmul(
                                ctx[qh][:, off:QROWS],
                                v_sb[kvh][:, g * 128:(g + 1) * 128],
                                exps[qh][:, 0:ng],
                                start=(g == 0), stop=(g == 15),
                                skip_group_check=True)
                    for qh in range(4):
                        nc.tensor.matmul(sums[:], sel4r[qh][:], acc[qh][:],
                                         start=(qh == 0), stop=(qh == 3),
                                         skip_group_check=True)
                    recip = npool.tile([4, QROWS], mybir.dt.float32r, tag="recip", name=f"recip{kvh}")
                    with nc.allow_low_precision(reason="f32r softmax denominators"):
                        nc.vector.reciprocal(recip[:], sums[:])
                    for qh in range(4):
                        h = kvh * 4 + qh
                        bps = bc_ps.tile([HD, QROWS], f32, tag="bc", name=f"bps{kvh}_{qh}")
                        nc.tensor.matmul(bps[:], sel128r[qh], recip[:],
                                         start=True, stop=True)
                        bsb = npool.tile([HD, QROWS], f32, tag="bsb", name=f"bsb{kvh}_{qh}")
                        nc.scalar.copy(bsb[:], bps[:])
                        nc.vector.tensor_mul(ctxn_sb[h][:], ctx[qh][:], bsb[:])

            # ---------------- phase 3: output projection ----------------
            with tc.tile_pool(name="out_ps", bufs=4, space="PSUM") as out_ps, \
                 tc.tile_pool(name="out_sb_pool", bufs=2) as osb_pool:
                for c in range(4):
                    osb = osb_pool.tile([128, D], f32, tag="osb", name=f"osb{c}")
                    for sl in range(4):
                        ps = out_ps.tile([128, 512], f32, tag="ops", name=f"ops{c}_{sl}")
                        for h in range(HQ):
                            nc.tensor.matmul(
                                ps[:],
                                ctxn_sb[h][:, c * 128:(c + 1) * 128],
                                wo_sb[h][:, sl * 512:(sl + 1) * 512],
                                start=(h == 0), stop=(h == HQ - 1))
                        nc.scalar.copy(osb[:, sl * 512:(sl + 1) * 512], ps[:])
                    nc.sync.dma_start(out[c * 128:(c + 1) * 128, :], osb[:])
            wop_cm.__exit__(None, None, None)

    nc.compile()
    return nc


def _prep_core_inputs(c, Xq, Xkv, wq2, wk2, wv2, wo2, q_positions, kv_positions):
    bfl = ml_dtypes.bfloat16
    b, j = divmod(c, 4)
    chunks = CHUNKS[j]
    qrows = np.concatenate([np.arange(64 * ch, 64 * ch + 64) for ch in chunks])
    kvrows = np.arange(512 * j, 512 * j + 512)

    inv_freq = 1.0 / (MAX_TIMESCALE **
                      (2.0 * np.arange(HALF, dtype=np.float32) / HD))
    pq = q_positions[b][qrows].astype(np.float32)
    pk = kv_positions[b][kvrows].astype(np.float32)
    fq = inv_freq[:, None] * pq[None, :]
    fk = inv_freq[:, None] * pk[None, :]

    # validity mask for the first 64-col sub-block of each kv block:
    # chunk i0 = g//2, columns are rows 64*c0..64*c0+63, valid iff kv <= q
    dm = np.zeros((16, 128, 64), dtype=np.float32)
    for g in range(16):
        c0 = chunks[g // 2]
        kv_idx = 128 * g + np.arange(128)[:, None]
        q_idx = 64 * c0 + np.arange(64)[None, :]
        dm[g] = (kv_idx <= q_idx).astype(np.float32)
    dm = np.tile(dm[:, :, None, :], (1, 1, 4, 1)).reshape(16, 128, 256)

    xq_dev = np.ascontiguousarray(
        Xq[b][qrows, :].T.reshape(16, 128, QROWS).transpose(1, 0, 2)
        .reshape(128, 16 * QROWS))
    xkv_dev = np.ascontiguousarray(
        Xkv[b][kvrows, :].T.reshape(16, 128, KVSH).transpose(1, 0, 2)
        .reshape(128, 16 * KVSH))
    return dict(
        xqT=xq_dev.astype(bfl),
        xkvT=xkv_dev.astype(bfl),
        wq=wq2, wk=wk2, wv=wv2, wo=wo2,
        cosq=np.concatenate([np.cos(fq)] * 2, axis=0).astype(bfl),
        sinq=np.concatenate([np.sin(fq)] * 2, axis=0).astype(bfl),
        coskv=np.concatenate([np.cos(fk)] * 2, axis=0).astype(bfl),
        sinkv=np.concatenate([np.sin(fk)] * 2, axis=0).astype(bfl),
        dmask=dm.astype(bfl),
        selbc=_selbc(),
    )


def _selbc():
    s = np.zeros((4, 4 * HD), dtype=np.float32)
    for q in range(4):
        s[q, q * HD:(q + 1) * HD] = 1.0
    return s


def kernel(Xq, Xkv, Wq, Wk, Wv, Wo, q_positions, kv_positions):
    from concourse import bass_utils

    Xq = np.asarray(Xq, dtype=np.float32)
    Xkv = np.asarray(Xkv, dtype=np.float32)
    Wq = np.asarray(Wq, dtype=np.float32)
    Wk = np.asarray(Wk, dtype=np.float32)
    Wv = np.asarray(Wv, dtype=np.float32)
    Wo = np.asarray(Wo, dtype=np.float32)
    q_positions = np.asarray(q_positions)
    kv_positions = np.asarray(kv_positions)

    if "nc" not in _CACHE:
        _CACHE["nc"] = _build()
    nc = _CACHE["nc"]

    bfl = ml_dtypes.bfloat16
    # wq: [128 p, h*16kt*128] so each head's lhsT block is contiguous
    wq2 = np.ascontiguousarray(
        Wq.reshape(16, 128, HQ, HD).transpose(1, 2, 0, 3)
        .reshape(128, HQ * 16 * HD)).astype(bfl)
    wk2 = np.ascontiguousarray(
        Wk.reshape(16, 128, HKV * HD).transpose(1, 0, 2)
        .reshape(128, 16 * HKV * HD)).astype(bfl)
    wv2 = np.ascontiguousarray(
        Wv.reshape(16, 128, HKV * HD).transpose(1, 0, 2)
        .reshape(128, 16 * HKV * HD)).astype(bfl)
    wo2 = np.ascontiguousarray(Wo.reshape(HQ * HD, D)).astype(bfl)

    in_maps = [_prep_core_inputs(c, Xq, Xkv, wq2, wk2, wv2, wo2,
                                 q_positions, kv_positions)
               for c in range(N_CORES)]

    res = bass_utils.run_bass_kernel_spmd(
        nc, in_maps, core_ids=list(range(N_CORES)),
        **_CACHE.get("run_kwargs", {}))
    _CACHE["last_results"] = res

    out = np.empty((B, T, D), dtype=np.float32)
    for c in range(N_CORES):
        b, j = divmod(c, 4)
        core_out = res.results[c]["out"]
        for i, ch in enumerate(CHUNKS[j]):
            out[b, 64 * ch:64 * ch + 64, :] = core_out[64 * i:64 * i + 64, :]
    return out

